# revision 19
# baseline (speedup 1.0000x reference)
"""MixLinear int4-GEMM kernel for 8x TRN2 NeuronCores.

Strategy: 2D sharding, 4 M-groups x 2 OUT-groups (each core owns 2048 rows
of x and 2048 output channels).  Host-side layout work (index shuffling
and exact dtype relabeling only):

  * The IN dimension is permuted so the 256 outlier columns are the last
    256 device columns.  The masked abs-max becomes a plain reduce over
    device cols [0:3840], and the outlier gather becomes a slice.
  * int4 weights are converted host-side to fp8e4m3 bit patterns via a
    16-entry LUT (exact: ints in [-8,7]) and laid out as [128, 30, OUT]
    so they DMA straight into the SBUF moving-operand tile - no on-device
    unpack at all.
  * weight_cache is host-transposed, divided by scale_col, and converted
    to bf16 bits; scale_col is converted to bf16 bits.  These DMA
    straight into their SBUF tiles.

Per core, per 128-row tile:
  1. DVE abs-max over x[:, :3840] -> s = max/7, r = 1/s.
  2. ScalarE magic round: bf16(x*r + 192) rounds to integer (bf16 ulp=1
     in [184,200)); DMA-xbar transpose; DVE -192 -> qT fp8e4 (exact).
  3. Outliers: ScalarE ao*r -> bf16, DMA-xbar transpose.
  4. 15 fp8 DoubleRow matmuls (256-deep each) + 2 bf16 outlier matmuls
     per 512-wide psum group accumulate into one [128, 2048] psum.
     Only the first matmul of each stationary-operand group issues
     LDWEIGHTS (see strip_redundant_ldweights).
  5. Dequant (pipelined one tile behind): ScalarE psum*s -> bf16,
     DVE *scale_col(bf16) -> y bf16.

Host assembles the 4x2 grid of [2048, 2048] bf16 shards into fp32.
"""

import numpy as np

B, S, IN, OUT, FP = 4, 2048, 4096, 4096, 256
M = B * S
NCORES = 8
MGROUPS, OGROUPS = 4, 2
MS = M // MGROUPS     # 2048 rows per core
OS = OUT // OGROUPS   # 2048 out-channels per core
KI = IN - FP          # 3840 int-path contraction cols
KT = KI // 128        # 30 int contraction chunks
FT = FP // 128        # 2 outlier chunks
QMAX = 7.0
MAGIC = 192.0         # 1.5 * 2**7: bf16 output rounding forces RNE to integer

# fp8e4m3 (bias 7) bit patterns for nibble codes 0..15 (two's complement
# int4 values 0..7, -8..-1).  Exact: all are normal numbers.
FP8_LUT = np.array(
    [0x00, 0x38, 0x40, 0x44, 0x48, 0x4A, 0x4C, 0x4E,
     0xD0, 0xCE, 0xCC, 0xCA, 0xC8, 0xC4, 0xC0, 0xB8],
    dtype=np.uint8,
)


def _bf16_bits(a):
    """float32 -> bf16 bit pattern (round to nearest even), as uint16."""
    b = np.ascontiguousarray(a, dtype=np.float32).view(np.uint32)
    return ((b + 0x7FFF + ((b >> 16) & 1)) >> 16).astype(np.uint16)


def emit_core_kernel(nc, tc, ms, os_dim, reuse_names):
    """Emit the per-core tile program. All dims compile-time constants."""
    import concourse.mybir as mybir
    import bass_rust

    f32 = mybir.dt.float32
    bf16 = mybir.dt.bfloat16
    u8 = mybir.dt.uint8
    u16 = mybir.dt.uint16
    fp8 = mybir.dt.float8e4
    Alu = mybir.AluOpType
    Act = mybir.ActivationFunctionType
    DR = mybir.MatmulPerfMode.DoubleRow

    P = 128
    MT = ms // P          # 16 activation tiles
    OJ = os_dim // 512    # 4 psum column groups

    x = nc.dram_tensor("x", [ms, IN], f32, kind="ExternalInput")
    qw8 = nc.dram_tensor("qw8", [P, KT, os_dim], u8, kind="ExternalInput")
    wcs16 = nc.dram_tensor("wcs16", [P, FT, os_dim], u16, kind="ExternalInput")
    scb16 = nc.dram_tensor("scb16", [os_dim], u16, kind="ExternalInput")
    y = nc.dram_tensor("y", [ms, os_dim], bf16, kind="ExternalOutput")

    with (
        tc.tile_pool(name="wp", bufs=1) as wp,
        tc.tile_pool(name="xp", bufs=4) as xp,
        tc.tile_pool(name="qp", bufs=3) as qp,
        tc.tile_pool(name="qtp", bufs=3) as qtp,
        tc.tile_pool(name="ftp", bufs=3) as ftp,
        tc.tile_pool(name="aop", bufs=3) as aop,
        tc.tile_pool(name="sp", bufs=8) as sp,
        tc.tile_pool(name="yp", bufs=2) as yp,
        tc.tile_pool(name="py", bufs=2, space="PSUM") as py,
    ):
        # ---------------- persistent weights ----------------
        wT = wp.tile([P, KT, os_dim], fp8)          # int4 weights, fp8 ints
        wcs = wp.tile([P, FT, os_dim], bf16)        # weight_cache / scale_col
        scb = wp.tile([P, os_dim], bf16)            # scale_col broadcast

        # Per-engine nosync dependency chains pin each engine queue to
        # emission order.  Without them the tile scheduler reorders: it
        # parks evict ops (which wait on the previous tile's matmuls) at
        # the queue head, blocking the next tile's quantize chain, and it
        # hoists every x prefetch ahead of the transposes.
        dep_nosync = bass_rust.DependencyInfo(sync=False, no_sync=True)
        chain_tail = {}

        def chain(key, inst):
            prev_name = chain_tail.get(key)
            if prev_name is not None:
                inst.ins.add_dependency(prev_name, dep_nosync)
            chain_tail[key] = inst.ins.name
            return inst

        inv7 = float(np.float32(1.0) / np.float32(QMAX))

        # evict is software-pipelined one tile behind the matmuls; it
        # runs entirely on DVE (psum * s_t per-partition, then * scale_col)
        # + the gpsimd SWDGE ring, so the scalar engine never parks on a
        # wait for the previous tile's matmuls.
        def emit_evict(psum, s_t, mi):
            t1 = yp.tile([P, os_dim], bf16, tag="t1")
            chain(
                "dve",
                nc.vector.tensor_scalar(t1[:], psum[:], s_t[:], None, Alu.mult),
            )
            chain("dve", nc.vector.tensor_tensor(t1[:], t1[:], scb[:], Alu.mult))
            chain("gps", nc.gpsimd.dma_start(y[mi * P : (mi + 1) * P, :], t1[:]))

        prev = None

        # PE weight-register reuse: 4 consecutive matmuls (the oj loop)
        # share the same stationary operand, so only the first needs
        # LDWEIGHTS.  The legalizer unconditionally splits every matmul
        # into InstLdweights + InstMatmult; matmuls recorded in
        # `reuse_names` get their redundant InstLdweights stripped after
        # legalization (see strip_redundant_ldweights).  The "pe" chain
        # pins PE-queue order so a later loader can't be scheduled
        # between a loader and its reusing matmuls.
        def emit_mm(load_weights, *args, **kwargs):
            mm = nc.tensor.matmul(*args, **kwargs)
            if not load_weights:
                reuse_names.add(mm.ins.name)
            return chain("pe", mm)

        def emit_x_load(mi):
            # the SP HWDGE queue is dedicated to x loads: its dispatch
            # waits (descriptor-ring backpressure) then only ever delay
            # later x loads, never the transposes or evicts.
            x_t = xp.tile([P, IN], f32)
            chain("sp", nc.sync.dma_start(x_t[:], x[mi * P : (mi + 1) * P, :]))
            return x_t

        x_tiles = {}
        fronts = {}
        mids = {}

        def emit_front_dve(mi):
            """abs-max / scales for tile mi (DVE only)."""
            x_t = x_tiles[mi]
            mx = sp.tile([P, 1], f32, tag="mx")
            chain(
                "dve",
                nc.vector.tensor_reduce(
                    mx[:], x_t[:, :KI], mybir.AxisListType.X, Alu.max,
                    apply_absolute_value=True,
                ),
            )
            s_t = sp.tile([P, 1], f32, tag="s")
            chain("dve", nc.vector.tensor_scalar(s_t[:], mx[:], inv7, None, Alu.mult))
            r_t = sp.tile([P, 1], f32, tag="r")
            chain("dve", nc.vector.reciprocal(r_t[:], s_t[:]))
            fronts[mi] = [s_t, r_t, None, None]

        def emit_front_act(mi):
            """outlier-scale / quantize for tile mi (scalar engine)."""
            x_t = x_tiles.pop(mi)
            f = fronts[mi]
            r_t = f[1]
            aos = aop.tile([P, FP], bf16, tag="aos")
            chain(
                "act",
                nc.scalar.activation(aos[:], x_t[:, KI:], Act.Copy, scale=r_t[:]),
            )
            # quantize: q+MAGIC = bf16(x*r + MAGIC) — the bf16 output convert
            # rounds to integer (ulp=1 in [184,200)); -MAGIC folds into the
            # fp8 convert after the transpose.
            q = qp.tile([P, KI], bf16)
            chain(
                "act",
                nc.scalar.activation(
                    q[:], x_t[:, :KI], Act.Copy, bias=MAGIC, scale=r_t[:]
                ),
            )
            f[2], f[3] = aos, q

        def emit_mid(mi):
            """transposes (scalar HWDGE) + fp8 fixup (DVE) for tile mi."""
            s_t, r_t, aos, q = fronts.pop(mi)
            aoT = aop.tile([P, FT, P], bf16, tag="aoT")
            chain("act", nc.scalar.dma_start_transpose(aoT[:], aos[:]))
            qTb = qtp.tile([P, KT, P], bf16)
            chain("act", nc.scalar.dma_start_transpose(qTb[:], q[:]))
            qT = ftp.tile([P, KT, P], fp8)
            chain("dve", nc.vector.tensor_scalar(qT[:], qTb[:], -MAGIC, None, Alu.add))
            mids[mi] = (s_t, aoT, qT)

        # Weight DMAs: bulk on the gpsimd SWDGE ring (idle otherwise),
        # the last chunks ride the scalar HWDGE ring ahead of any
        # activation compute, ordered so chunks land just ahead of the
        # matmuls that consume them.  x loads own the SP ring.
        x_tiles[0] = emit_x_load(0)
        x_tiles[1] = emit_x_load(1)
        for j0, j1 in ((16, 24), (24, KT)):
            chain(
                "act",
                nc.scalar.dma_start(wT[:, j0:j1, :].bitcast(u8), qw8[:, j0:j1, :]),
            )
        for j0, j1 in ((0, 2), (2, 8), (8, 16)):
            chain(
                "gps",
                nc.gpsimd.dma_start(wT[:, j0:j1, :].bitcast(u8), qw8[:, j0:j1, :]),
            )
        chain("gps", nc.gpsimd.dma_start(wcs[:].bitcast(u16), wcs16[:, :, :]))
        chain(
            "gps",
            nc.gpsimd.dma_start(
                scb[:].bitcast(u16), scb16[None, :].to_broadcast((P, os_dim))
            ),
        )

        # ---------------- software-pipelined main loop ----------------
        # iteration k runs: front_dve(k+2) | mid(k+1) | front_act(k+2) |
        # x-prefetch(k+4) | matmuls(k) | evict(k-1).  The front/mid skew
        # keeps the quantize chain (~17us latency) two phases ahead of
        # the PE; per-engine emission order keeps urgent ops ahead of
        # ones that park on waits.
        emit_front_dve(0)
        emit_front_act(0)
        emit_front_dve(1)
        emit_front_act(1)
        emit_mid(0)
        x_tiles[2] = emit_x_load(2)
        x_tiles[3] = emit_x_load(3)

        prev = None
        for mi in range(MT):
            if mi + 2 < MT:
                emit_front_dve(mi + 2)
            if mi + 1 < MT:
                emit_mid(mi + 1)
            if mi + 2 < MT:
                emit_front_act(mi + 2)
            if mi + 4 < MT:
                x_tiles[mi + 4] = emit_x_load(mi + 4)

            s_t, aoT, qT = mids.pop(mi)
            # GEMM: 15 int + 2 outlier matmuls per 512 group
            psum = py.tile([P, os_dim], f32)
            for c in range(KT // 2):
                for oj in range(OJ):
                    emit_mm(
                        oj == 0,
                        psum[:, oj * 512 : (oj + 1) * 512],
                        qT[:, 2 * c : 2 * c + 2, :],
                        wT[:, 2 * c : 2 * c + 2, oj * 512 : (oj + 1) * 512],
                        start=(c == 0),
                        stop=False,
                        perf_mode=DR,
                    )
            for f in range(FT):
                for oj in range(OJ):
                    emit_mm(
                        oj == 0,
                        psum[:, oj * 512 : (oj + 1) * 512],
                        aoT[:, f, :],
                        wcs[:, f, oj * 512 : (oj + 1) * 512],
                        start=False,
                        stop=(f == FT - 1),
                    )

            if prev is not None:
                emit_evict(*prev)
            prev = (psum, s_t, mi)

        emit_evict(*prev)

    return nc


def strip_redundant_ldweights(nc, reuse_names):
    """Delete InstLdweights whose matmult reuses the already-loaded PE
    weights.  Runs after tile legalization (which pairs each matmult with
    its own InstLdweights, inserted immediately before it in the block)
    and before bacc compile.  An LDW is removed only when (a) the next PE
    instruction is a matmult flagged for reuse, (b) its weights AP is
    byte-identical to the most recent retained LDW on the PE stream, and
    (c) it carries no semaphore waits/updates."""
    import concourse.mybir as mybir

    def ap_key(pap):
        return (pap.memref, pap.offset, str(pap.ap), str(pap.dtype))

    removed = kept = 0
    for fn in nc.m.functions:
        for bb in fn.blocks:
            insts = list(bb.instructions)
            pe_next = {}  # idx -> next PE instruction
            nxt = None
            for idx in range(len(insts) - 1, -1, -1):
                pe_next[idx] = nxt
                if insts[idx].engine == mybir.EngineType.PE:
                    nxt = insts[idx]
            keep = []
            last_w = None
            changed = False
            for idx, inst in enumerate(insts):
                if isinstance(inst, mybir.InstLdweights):
                    w = ap_key(inst.ins[0])
                    mm = pe_next[idx]
                    si = inst.sync_info
                    si_clear = si is None or (
                        len(si.on_wait) == 0 and len(si.on_update) == 0
                    )
                    if (
                        isinstance(mm, mybir.InstMatmult)
                        and mm.name in reuse_names
                        and w == last_w
                        and si_clear
                    ):
                        removed += 1
                        changed = True
                        continue
                    if isinstance(mm, mybir.InstMatmult) and mm.name in reuse_names:
                        kept += 1
                    last_w = w
                keep.append(inst)
            if changed:
                bb.instructions = keep
    return removed, kept


def build_nc(ms=MS, os_dim=OS):
    import concourse.bacc as bacc
    import concourse.tile as tile

    nc = bacc.Bacc(None, target_bir_lowering=False)
    reuse_names = set()
    with tile.TileContext(nc) as tc:
        emit_core_kernel(nc, tc, ms, os_dim, reuse_names)
    removed, kept = strip_redundant_ldweights(nc, reuse_names)
    assert removed > 0, f"ldweights strip removed nothing (kept={kept})"
    nc.compile()
    return nc


def make_host_inputs(x, q_weight, scale_col, weight_cache, ind,
                     ms=MS, os_dim=OS, ncores=NCORES):
    """Shard/relayout full inputs into per-core input maps.

    Host work is index shuffling plus exact dtype relabeling (int4 codes
    -> fp8 bit patterns) and the weight_cache/scale_col division +
    bf16 rounding (weight preprocessing identical to what the device
    previously computed)."""
    ind = np.asarray(ind).astype(np.int64)
    notout = np.setdiff1d(np.arange(IN, dtype=np.int64), ind)   # 3840 sorted
    perm = np.concatenate([notout, ind])                        # dev col -> orig

    xf = np.asarray(x).reshape(M, IN).astype(np.float32, copy=False)
    xp = np.ascontiguousarray(xf[:, perm])                      # [M, IN]

    v = np.asarray(q_weight).astype(np.uint8)                   # [OUT, IN//2]
    nib = np.empty((OUT, IN), dtype=np.uint8)                   # nibble codes
    nib[:, 0::2] = v & 15
    nib[:, 1::2] = v >> 4
    nibp = nib[:, perm[:KI]]                                    # [OUT, KI]
    w8 = FP8_LUT[nibp]                                          # fp8 bits
    # device layout [p, j, o]: contraction index k = j*128 + p
    qw8 = np.ascontiguousarray(
        w8.T.reshape(KT, 128, OUT).transpose(1, 0, 2)
    )                                                           # [128, KT, OUT]

    scf = np.asarray(scale_col).reshape(-1).astype(np.float32, copy=False)
    wcT = np.asarray(weight_cache).astype(np.float32, copy=False).T  # [FP, OUT]
    wcs16 = _bf16_bits(wcT / scf[None, :]).reshape(FT, 128, OUT).transpose(1, 0, 2)
    wcs16 = np.ascontiguousarray(wcs16)                         # [128, FT, OUT]
    scb16 = _bf16_bits(scf)                                     # [OUT]

    in_maps = []
    for c in range(ncores):
        mg, og = divmod(c, OGROUPS)
        m0, o0 = mg * ms, og * os_dim
        in_maps.append(
            {
                "x": xp[m0 : m0 + ms],
                "qw8": np.ascontiguousarray(qw8[:, :, o0 : o0 + os_dim]),
                "wcs16": np.ascontiguousarray(wcs16[:, :, o0 : o0 + os_dim]),
                "scb16": np.ascontiguousarray(scb16[o0 : o0 + os_dim]),
            }
        )
    return in_maps


_NC_CACHE = {}


def kernel(x, q_weight, scale_col, weight_cache, ind, trace=False):
    from concourse.bass_utils import run_bass_kernel_spmd

    key = "full"
    if key not in _NC_CACHE:
        _NC_CACHE[key] = build_nc()
    nc = _NC_CACHE[key]

    in_maps = make_host_inputs(x, q_weight, scale_col, weight_cache, ind)
    res = run_bass_kernel_spmd(nc, in_maps, list(range(NCORES)), trace=trace)
    yfull = np.empty((M, OUT), dtype=np.float32)
    for c in range(NCORES):
        mg, og = divmod(c, OGROUPS)
        yfull[mg * MS : (mg + 1) * MS, og * OS : (og + 1) * OS] = np.asarray(
            res.results[c]["y"]
        ).astype(np.float32)
    yfull = yfull.reshape(B, S, OUT)
    if trace:
        return yfull, res
    return yfull


# revision 22
# speedup vs baseline: 1.1152x; 1.1152x over previous
"""MixLinear int4-GEMM kernel for 8x TRN2 NeuronCores.

Strategy: 2D sharding, 4 M-groups x 2 OUT-groups (each core owns 2048 rows
of x and 2048 output channels).  Host-side layout work (index shuffling
and exact dtype relabeling only):

  * The IN dimension is permuted so the 256 outlier columns are the last
    256 device columns.  The masked abs-max becomes a plain reduce over
    device cols [0:3840], and the outlier gather becomes a slice.
  * int4 weights are converted host-side to fp8e4m3 bit patterns via a
    16-entry LUT (exact: ints in [-8,7]) and laid out as [128, 30, OUT]
    so they DMA straight into the SBUF moving-operand tile - no on-device
    unpack at all.
  * weight_cache is host-transposed, divided by scale_col, and converted
    to bf16 bits; scale_col is converted to bf16 bits.  These DMA
    straight into their SBUF tiles.

Per core, per 128-row tile:
  1. DVE abs-max over x[:, :3840] -> s = max/7, r = 1/s.
  2. ScalarE magic round: bf16(x*r + 192) rounds to integer (bf16 ulp=1
     in [184,200)); DMA-xbar transpose; DVE -192 -> qT fp8e4 (exact).
  3. Outliers: ScalarE ao*r -> bf16, DMA-xbar transpose.
  4. 15 fp8 DoubleRow matmuls (256-deep each) + 2 bf16 outlier matmuls
     per 512-wide psum group accumulate into one [128, 2048] psum.
     Only the first matmul of each stationary-operand group issues
     LDWEIGHTS (see strip_redundant_ldweights).
  5. Dequant (pipelined one tile behind): ScalarE psum*s -> bf16,
     DVE *scale_col(bf16) -> y bf16.

Host assembles the 4x2 grid of [2048, 2048] bf16 shards into fp32.
"""

import numpy as np

B, S, IN, OUT, FP = 4, 2048, 4096, 4096, 256
M = B * S
NCORES = 8
MGROUPS, OGROUPS = 4, 2
MS = M // MGROUPS     # 2048 rows per core
OS = OUT // OGROUPS   # 2048 out-channels per core
KI = IN - FP          # 3840 int-path contraction cols
KT = KI // 128        # 30 int contraction chunks
FT = FP // 128        # 2 outlier chunks
QMAX = 7.0
MAGIC = 192.0         # 1.5 * 2**7: bf16 output rounding forces RNE to integer

# fp8e4m3 (bias 7) bit patterns for nibble codes 0..15 (two's complement
# int4 values 0..7, -8..-1).  Exact: all are normal numbers.
FP8_LUT = np.array(
    [0x00, 0x38, 0x40, 0x44, 0x48, 0x4A, 0x4C, 0x4E,
     0xD0, 0xCE, 0xCC, 0xCA, 0xC8, 0xC4, 0xC0, 0xB8],
    dtype=np.uint8,
)


def _bf16_bits(a):
    """float32 -> bf16 bit pattern (round to nearest even), as uint16."""
    b = np.ascontiguousarray(a, dtype=np.float32).view(np.uint32)
    return ((b + 0x7FFF + ((b >> 16) & 1)) >> 16).astype(np.uint16)


def emit_core_kernel(nc, tc, ms, os_dim, reuse_names):
    """Emit the per-core tile program. All dims compile-time constants."""
    import concourse.mybir as mybir
    import bass_rust

    f32 = mybir.dt.float32
    bf16 = mybir.dt.bfloat16
    u8 = mybir.dt.uint8
    u16 = mybir.dt.uint16
    fp8 = mybir.dt.float8e4
    Alu = mybir.AluOpType
    Act = mybir.ActivationFunctionType
    DR = mybir.MatmulPerfMode.DoubleRow

    P = 128
    MT = ms // P          # 16 activation tiles
    OJ = os_dim // 512    # 4 psum column groups

    x = nc.dram_tensor("x", [ms, IN], f32, kind="ExternalInput")
    qw8 = nc.dram_tensor("qw8", [P, KT, os_dim], u8, kind="ExternalInput")
    wcs16 = nc.dram_tensor("wcs16", [P, FT, os_dim], u16, kind="ExternalInput")
    scb16 = nc.dram_tensor("scb16", [os_dim], u16, kind="ExternalInput")
    y = nc.dram_tensor("y", [ms, os_dim], bf16, kind="ExternalOutput")

    with (
        tc.tile_pool(name="wp", bufs=1) as wp,
        tc.tile_pool(name="xp", bufs=4) as xp,
        tc.tile_pool(name="qp", bufs=3) as qp,
        tc.tile_pool(name="qtp", bufs=3) as qtp,
        tc.tile_pool(name="ftp", bufs=3) as ftp,
        tc.tile_pool(name="aop", bufs=3) as aop,
        tc.tile_pool(name="sp", bufs=8) as sp,
        tc.tile_pool(name="yp", bufs=2) as yp,
        tc.tile_pool(name="py", bufs=2, space="PSUM") as py,
    ):
        # ---------------- persistent weights ----------------
        wT = wp.tile([P, KT, os_dim], fp8)          # int4 weights, fp8 ints
        wcs = wp.tile([P, FT, os_dim], bf16)        # weight_cache / scale_col
        scb = wp.tile([P, os_dim], bf16)            # scale_col broadcast

        # Per-engine nosync dependency chains pin each engine queue to
        # emission order.  Without them the tile scheduler reorders: it
        # parks evict ops (which wait on the previous tile's matmuls) at
        # the queue head, blocking the next tile's quantize chain, and it
        # hoists every x prefetch ahead of the transposes.
        dep_nosync = bass_rust.DependencyInfo(sync=False, no_sync=True)
        chain_tail = {}

        def chain(key, inst):
            prev_name = chain_tail.get(key)
            if prev_name is not None:
                inst.ins.add_dependency(prev_name, dep_nosync)
            chain_tail[key] = inst.ins.name
            return inst

        inv7 = float(np.float32(1.0) / np.float32(QMAX))

        # evict is software-pipelined one tile behind the matmuls; it
        # runs entirely on DVE (psum * s_t per-partition, then * scale_col)
        # + the gpsimd SWDGE ring, so the scalar engine never parks on a
        # wait for the previous tile's matmuls.
        def emit_evict(psum, s_t, mi):
            t1 = yp.tile([P, os_dim], bf16, tag="t1")
            chain(
                "dve",
                nc.vector.tensor_scalar(t1[:], psum[:], s_t[:], None, Alu.mult),
            )
            chain("dve", nc.vector.tensor_tensor(t1[:], t1[:], scb[:], Alu.mult))
            chain("act", nc.scalar.dma_start(y[mi * P : (mi + 1) * P, :], t1[:]))

        prev = None

        # PE weight-register reuse: 4 consecutive matmuls (the oj loop)
        # share the same stationary operand, so only the first needs
        # LDWEIGHTS.  The legalizer unconditionally splits every matmul
        # into InstLdweights + InstMatmult; matmuls recorded in
        # `reuse_names` get their redundant InstLdweights stripped after
        # legalization (see strip_redundant_ldweights).  The "pe" chain
        # pins PE-queue order so a later loader can't be scheduled
        # between a loader and its reusing matmuls.
        def emit_mm(load_weights, *args, **kwargs):
            mm = nc.tensor.matmul(*args, **kwargs)
            if not load_weights:
                reuse_names.add(mm.ins.name)
            return chain("pe", mm)

        def emit_x_load(mi):
            # the SP HWDGE queue is dedicated to x loads: its dispatch
            # waits (descriptor-ring backpressure) then only ever delay
            # later x loads, never the transposes or evicts.
            x_t = xp.tile([P, IN], f32)
            chain("sp", nc.sync.dma_start(x_t[:], x[mi * P : (mi + 1) * P, :]))
            return x_t

        x_tiles = {}
        fronts = {}
        mids = {}

        def emit_front_dve(mi):
            """abs-max / scales for tile mi (DVE only)."""
            x_t = x_tiles[mi]
            mx = sp.tile([P, 1], f32, tag="mx")
            chain(
                "dve",
                nc.vector.tensor_reduce(
                    mx[:], x_t[:, :KI], mybir.AxisListType.X, Alu.max,
                    apply_absolute_value=True,
                ),
            )
            s_t = sp.tile([P, 1], f32, tag="s")
            chain("dve", nc.vector.tensor_scalar(s_t[:], mx[:], inv7, None, Alu.mult))
            r_t = sp.tile([P, 1], f32, tag="r")
            chain("dve", nc.vector.reciprocal(r_t[:], s_t[:]))
            fronts[mi] = [s_t, r_t, None, None]

        def emit_front_act(mi):
            """outlier-scale / quantize for tile mi (scalar engine)."""
            x_t = x_tiles.pop(mi)
            f = fronts[mi]
            r_t = f[1]
            aos = aop.tile([P, FP], bf16, tag="aos")
            chain(
                "act",
                nc.scalar.activation(aos[:], x_t[:, KI:], Act.Copy, scale=r_t[:]),
            )
            # quantize: q+MAGIC = bf16(x*r + MAGIC) — the bf16 output convert
            # rounds to integer (ulp=1 in [184,200)); -MAGIC folds into the
            # fp8 convert after the transpose.
            q = qp.tile([P, KI], bf16)
            chain(
                "act",
                nc.scalar.activation(
                    q[:], x_t[:, :KI], Act.Copy, bias=MAGIC, scale=r_t[:]
                ),
            )
            f[2], f[3] = aos, q

        def emit_mid(mi):
            """transposes (scalar HWDGE) + fp8 fixup (DVE) for tile mi."""
            s_t, r_t, aos, q = fronts.pop(mi)
            aoT = aop.tile([P, FT, P], bf16, tag="aoT")
            chain("act", nc.scalar.dma_start_transpose(aoT[:], aos[:]))
            qTb = qtp.tile([P, KT, P], bf16)
            chain("act", nc.scalar.dma_start_transpose(qTb[:], q[:]))
            qT = ftp.tile([P, KT, P], fp8)
            chain("dve", nc.vector.tensor_scalar(qT[:], qTb[:], -MAGIC, None, Alu.add))
            mids[mi] = (s_t, aoT, qT)

        # Weight DMAs: bulk on the gpsimd SWDGE ring (idle otherwise),
        # the last chunks ride the scalar HWDGE ring ahead of any
        # activation compute, ordered so chunks land just ahead of the
        # matmuls that consume them.  x loads own the SP ring.
        x_tiles[0] = emit_x_load(0)
        x_tiles[1] = emit_x_load(1)
        for j0, j1 in ((16, 24), (24, KT)):
            chain(
                "act",
                nc.scalar.dma_start(wT[:, j0:j1, :].bitcast(u8), qw8[:, j0:j1, :]),
            )
        for j0, j1 in ((0, 2), (2, 8), (8, 16)):
            chain(
                "gps",
                nc.gpsimd.dma_start(wT[:, j0:j1, :].bitcast(u8), qw8[:, j0:j1, :]),
            )
        chain("gps", nc.gpsimd.dma_start(wcs[:].bitcast(u16), wcs16[:, :, :]))
        chain(
            "gps",
            nc.gpsimd.dma_start(
                scb[:].bitcast(u16), scb16[None, :].to_broadcast((P, os_dim))
            ),
        )

        # ---------------- software-pipelined main loop ----------------
        # iteration k runs: front_dve(k+2) | mid(k+1) | front_act(k+2) |
        # x-prefetch(k+4) | matmuls(k) | evict(k-1).  The front/mid skew
        # keeps the quantize chain (~17us latency) two phases ahead of
        # the PE; per-engine emission order keeps urgent ops ahead of
        # ones that park on waits.
        emit_front_dve(0)
        emit_front_act(0)
        emit_front_dve(1)
        emit_front_act(1)
        emit_mid(0)
        x_tiles[2] = emit_x_load(2)
        x_tiles[3] = emit_x_load(3)

        prev = None
        for mi in range(MT):
            if mi + 2 < MT:
                emit_front_dve(mi + 2)
            if mi + 1 < MT:
                emit_mid(mi + 1)
            if mi + 2 < MT:
                emit_front_act(mi + 2)
            if mi + 4 < MT:
                x_tiles[mi + 4] = emit_x_load(mi + 4)

            s_t, aoT, qT = mids.pop(mi)
            # GEMM: 15 int + 2 outlier matmuls per 512 group
            psum = py.tile([P, os_dim], f32)
            for c in range(KT // 2):
                for oj in range(OJ):
                    emit_mm(
                        oj == 0,
                        psum[:, oj * 512 : (oj + 1) * 512],
                        qT[:, 2 * c : 2 * c + 2, :],
                        wT[:, 2 * c : 2 * c + 2, oj * 512 : (oj + 1) * 512],
                        start=(c == 0),
                        stop=False,
                        perf_mode=DR,
                    )
            for f in range(FT):
                for oj in range(OJ):
                    emit_mm(
                        oj == 0,
                        psum[:, oj * 512 : (oj + 1) * 512],
                        aoT[:, f, :],
                        wcs[:, f, oj * 512 : (oj + 1) * 512],
                        start=False,
                        stop=(f == FT - 1),
                    )

            if prev is not None:
                emit_evict(*prev)
            prev = (psum, s_t, mi)

        emit_evict(*prev)

    return nc


def strip_redundant_ldweights(nc, reuse_names):
    """Delete InstLdweights whose matmult reuses the already-loaded PE
    weights.  Runs after tile legalization (which pairs each matmult with
    its own InstLdweights, inserted immediately before it in the block)
    and before bacc compile.  An LDW is removed only when (a) the next PE
    instruction is a matmult flagged for reuse, (b) its weights AP is
    byte-identical to the most recent retained LDW on the PE stream, and
    (c) it carries no semaphore waits/updates."""
    import concourse.mybir as mybir

    def ap_key(pap):
        return (pap.memref, pap.offset, str(pap.ap), str(pap.dtype))

    removed = kept = 0
    for fn in nc.m.functions:
        for bb in fn.blocks:
            insts = list(bb.instructions)
            pe_next = {}  # idx -> next PE instruction
            nxt = None
            for idx in range(len(insts) - 1, -1, -1):
                pe_next[idx] = nxt
                if insts[idx].engine == mybir.EngineType.PE:
                    nxt = insts[idx]
            keep = []
            last_w = None
            changed = False
            for idx, inst in enumerate(insts):
                if isinstance(inst, mybir.InstLdweights):
                    w = ap_key(inst.ins[0])
                    mm = pe_next[idx]
                    si = inst.sync_info
                    si_clear = si is None or (
                        len(si.on_wait) == 0 and len(si.on_update) == 0
                    )
                    if (
                        isinstance(mm, mybir.InstMatmult)
                        and mm.name in reuse_names
                        and w == last_w
                        and si_clear
                    ):
                        removed += 1
                        changed = True
                        continue
                    if isinstance(mm, mybir.InstMatmult) and mm.name in reuse_names:
                        kept += 1
                    last_w = w
                keep.append(inst)
            if changed:
                bb.instructions = keep
    return removed, kept


def build_nc(ms=MS, os_dim=OS):
    import concourse.bacc as bacc
    import concourse.tile as tile

    nc = bacc.Bacc(None, target_bir_lowering=False)
    reuse_names = set()
    with tile.TileContext(nc) as tc:
        emit_core_kernel(nc, tc, ms, os_dim, reuse_names)
    removed, kept = strip_redundant_ldweights(nc, reuse_names)
    assert removed > 0, f"ldweights strip removed nothing (kept={kept})"
    nc.compile()
    return nc


def make_host_inputs(x, q_weight, scale_col, weight_cache, ind,
                     ms=MS, os_dim=OS, ncores=NCORES):
    """Shard/relayout full inputs into per-core input maps.

    Host work is index shuffling plus exact dtype relabeling (int4 codes
    -> fp8 bit patterns) and the weight_cache/scale_col division +
    bf16 rounding (weight preprocessing identical to what the device
    previously computed)."""
    ind = np.asarray(ind).astype(np.int64)
    notout = np.setdiff1d(np.arange(IN, dtype=np.int64), ind)   # 3840 sorted
    perm = np.concatenate([notout, ind])                        # dev col -> orig

    xf = np.asarray(x).reshape(M, IN).astype(np.float32, copy=False)
    xp = np.ascontiguousarray(xf[:, perm])                      # [M, IN]

    v = np.asarray(q_weight).astype(np.uint8)                   # [OUT, IN//2]
    nib = np.empty((OUT, IN), dtype=np.uint8)                   # nibble codes
    nib[:, 0::2] = v & 15
    nib[:, 1::2] = v >> 4
    nibp = nib[:, perm[:KI]]                                    # [OUT, KI]
    w8 = FP8_LUT[nibp]                                          # fp8 bits
    # device layout [p, j, o]: contraction index k = j*128 + p
    qw8 = np.ascontiguousarray(
        w8.T.reshape(KT, 128, OUT).transpose(1, 0, 2)
    )                                                           # [128, KT, OUT]

    scf = np.asarray(scale_col).reshape(-1).astype(np.float32, copy=False)
    wcT = np.asarray(weight_cache).astype(np.float32, copy=False).T  # [FP, OUT]
    wcs16 = _bf16_bits(wcT / scf[None, :]).reshape(FT, 128, OUT).transpose(1, 0, 2)
    wcs16 = np.ascontiguousarray(wcs16)                         # [128, FT, OUT]
    scb16 = _bf16_bits(scf)                                     # [OUT]

    in_maps = []
    for c in range(ncores):
        mg, og = divmod(c, OGROUPS)
        m0, o0 = mg * ms, og * os_dim
        in_maps.append(
            {
                "x": xp[m0 : m0 + ms],
                "qw8": np.ascontiguousarray(qw8[:, :, o0 : o0 + os_dim]),
                "wcs16": np.ascontiguousarray(wcs16[:, :, o0 : o0 + os_dim]),
                "scb16": np.ascontiguousarray(scb16[o0 : o0 + os_dim]),
            }
        )
    return in_maps


_NC_CACHE = {}


def kernel(x, q_weight, scale_col, weight_cache, ind, trace=False):
    from concourse.bass_utils import run_bass_kernel_spmd

    key = "full"
    if key not in _NC_CACHE:
        _NC_CACHE[key] = build_nc()
    nc = _NC_CACHE[key]

    in_maps = make_host_inputs(x, q_weight, scale_col, weight_cache, ind)
    res = run_bass_kernel_spmd(nc, in_maps, list(range(NCORES)), trace=trace)
    yfull = np.empty((M, OUT), dtype=np.float32)
    for c in range(NCORES):
        mg, og = divmod(c, OGROUPS)
        yfull[mg * MS : (mg + 1) * MS, og * OS : (og + 1) * OS] = np.asarray(
            res.results[c]["y"]
        ).astype(np.float32)
    yfull = yfull.reshape(B, S, OUT)
    if trace:
        return yfull, res
    return yfull


# revision 24
# speedup vs baseline: 1.1225x; 1.0065x over previous
"""MixLinear int4-GEMM kernel for 8x TRN2 NeuronCores.

Strategy: 2D sharding, 4 M-groups x 2 OUT-groups (each core owns 2048 rows
of x and 2048 output channels).  Host-side layout work (index shuffling
and exact dtype relabeling only):

  * The IN dimension is permuted so the 256 outlier columns are the last
    256 device columns.  The masked abs-max becomes a plain reduce over
    device cols [0:3840], and the outlier gather becomes a slice.
  * int4 weights are converted host-side to fp8e4m3 bit patterns via a
    16-entry LUT (exact: ints in [-8,7]) and laid out as [128, 30, OUT]
    so they DMA straight into the SBUF moving-operand tile - no on-device
    unpack at all.
  * weight_cache is host-transposed, divided by scale_col, and converted
    to bf16 bits; scale_col is converted to bf16 bits.  These DMA
    straight into their SBUF tiles.

Per core, per 128-row tile:
  1. DVE abs-max over x[:, :3840] -> s = max/7, r = 1/s.
  2. ScalarE magic round: bf16(x*r + 192) rounds to integer (bf16 ulp=1
     in [184,200)); DMA-xbar transpose; DVE -192 -> qT fp8e4 (exact).
  3. Outliers: ScalarE ao*r -> bf16, DMA-xbar transpose.
  4. 15 fp8 DoubleRow matmuls (256-deep each) + 2 bf16 outlier matmuls
     per 512-wide psum group accumulate into one [128, 2048] psum.
     Only the first matmul of each stationary-operand group issues
     LDWEIGHTS (see strip_redundant_ldweights).
  5. Dequant (pipelined one tile behind): ScalarE psum*s -> bf16,
     DVE *scale_col(bf16) -> y bf16.

Host assembles the 4x2 grid of [2048, 2048] bf16 shards into fp32.
"""

import numpy as np

B, S, IN, OUT, FP = 4, 2048, 4096, 4096, 256
M = B * S
NCORES = 8
MGROUPS, OGROUPS = 4, 2
MS = M // MGROUPS     # 2048 rows per core
OS = OUT // OGROUPS   # 2048 out-channels per core
KI = IN - FP          # 3840 int-path contraction cols
KT = KI // 128        # 30 int contraction chunks
FT = FP // 128        # 2 outlier chunks
QMAX = 7.0
MAGIC = 192.0         # 1.5 * 2**7: bf16 output rounding forces RNE to integer

# fp8e4m3 (bias 7) bit patterns for nibble codes 0..15 (two's complement
# int4 values 0..7, -8..-1).  Exact: all are normal numbers.
FP8_LUT = np.array(
    [0x00, 0x38, 0x40, 0x44, 0x48, 0x4A, 0x4C, 0x4E,
     0xD0, 0xCE, 0xCC, 0xCA, 0xC8, 0xC4, 0xC0, 0xB8],
    dtype=np.uint8,
)


def _bf16_bits(a):
    """float32 -> bf16 bit pattern (round to nearest even), as uint16."""
    b = np.ascontiguousarray(a, dtype=np.float32).view(np.uint32)
    return ((b + 0x7FFF + ((b >> 16) & 1)) >> 16).astype(np.uint16)


def emit_core_kernel(nc, tc, ms, os_dim, reuse_names):
    """Emit the per-core tile program. All dims compile-time constants."""
    import concourse.mybir as mybir
    import bass_rust

    f32 = mybir.dt.float32
    bf16 = mybir.dt.bfloat16
    u8 = mybir.dt.uint8
    u16 = mybir.dt.uint16
    fp8 = mybir.dt.float8e4
    Alu = mybir.AluOpType
    Act = mybir.ActivationFunctionType
    DR = mybir.MatmulPerfMode.DoubleRow

    P = 128
    MT = ms // P          # 16 activation tiles
    OJ = os_dim // 512    # 4 psum column groups

    x = nc.dram_tensor("x", [ms, IN], f32, kind="ExternalInput")
    qw8 = nc.dram_tensor("qw8", [P, KT, os_dim], u8, kind="ExternalInput")
    wcs16 = nc.dram_tensor("wcs16", [P, FT, os_dim], u16, kind="ExternalInput")
    scb16 = nc.dram_tensor("scb16", [os_dim], u16, kind="ExternalInput")
    y = nc.dram_tensor("y", [ms, os_dim], bf16, kind="ExternalOutput")

    with (
        tc.tile_pool(name="wp", bufs=1) as wp,
        tc.tile_pool(name="xp", bufs=4) as xp,
        tc.tile_pool(name="qp", bufs=3) as qp,
        tc.tile_pool(name="qtp", bufs=3) as qtp,
        tc.tile_pool(name="ftp", bufs=3) as ftp,
        tc.tile_pool(name="aop", bufs=3) as aop,
        tc.tile_pool(name="sp", bufs=8) as sp,
        tc.tile_pool(name="yp", bufs=2) as yp,
        tc.tile_pool(name="py", bufs=2, space="PSUM") as py,
    ):
        # ---------------- persistent weights ----------------
        wT = wp.tile([P, KT, os_dim], fp8)          # int4 weights, fp8 ints
        wcs = wp.tile([P, FT, os_dim], bf16)        # weight_cache / scale_col
        scb = wp.tile([P, os_dim], bf16)            # scale_col broadcast

        # Per-engine nosync dependency chains pin each engine queue to
        # emission order.  Without them the tile scheduler reorders: it
        # parks evict ops (which wait on the previous tile's matmuls) at
        # the queue head, blocking the next tile's quantize chain, and it
        # hoists every x prefetch ahead of the transposes.
        dep_nosync = bass_rust.DependencyInfo(sync=False, no_sync=True)
        chain_tail = {}

        def chain(key, inst):
            prev_name = chain_tail.get(key)
            if prev_name is not None:
                inst.ins.add_dependency(prev_name, dep_nosync)
            chain_tail[key] = inst.ins.name
            return inst

        inv7 = float(np.float32(1.0) / np.float32(QMAX))

        # evict is software-pipelined one tile behind the matmuls; it
        # runs entirely on DVE (psum * s_t per-partition, then * scale_col)
        # + the gpsimd SWDGE ring, so the scalar engine never parks on a
        # wait for the previous tile's matmuls.
        def emit_evict(psum, s_t, mi):
            t1 = yp.tile([P, os_dim], bf16, tag="t1")
            chain(
                "dve",
                nc.vector.tensor_scalar(t1[:], psum[:], s_t[:], None, Alu.mult),
            )
            chain("dve", nc.vector.tensor_tensor(t1[:], t1[:], scb[:], Alu.mult))
            chain("act", nc.scalar.dma_start(y[mi * P : (mi + 1) * P, :], t1[:]))

        prev = None

        # PE weight-register reuse: 4 consecutive matmuls (the oj loop)
        # share the same stationary operand, so only the first needs
        # LDWEIGHTS.  The legalizer unconditionally splits every matmul
        # into InstLdweights + InstMatmult; matmuls recorded in
        # `reuse_names` get their redundant InstLdweights stripped after
        # legalization (see strip_redundant_ldweights).  The "pe" chain
        # pins PE-queue order so a later loader can't be scheduled
        # between a loader and its reusing matmuls.
        def emit_mm(load_weights, *args, **kwargs):
            mm = nc.tensor.matmul(*args, **kwargs)
            if not load_weights:
                reuse_names.add(mm.ins.name)
            return chain("pe", mm)

        def emit_x_load(mi):
            # split row-halves across the two HWDGE rings (~6us each):
            # the SP ring also carries the transposes, the ACT ring the
            # y stores.
            x_t = xp.tile([P, IN], f32)
            chain("sp", nc.sync.dma_start(x_t[:64], x[mi * P : mi * P + 64, :]))
            chain(
                "act",
                nc.scalar.dma_start(x_t[64:], x[mi * P + 64 : (mi + 1) * P, :]),
            )
            return x_t

        x_tiles = {}
        fronts = {}
        mids = {}

        def emit_front_dve(mi):
            """abs-max / scales for tile mi (DVE only)."""
            x_t = x_tiles[mi]
            mx = sp.tile([P, 1], f32, tag="mx")
            chain(
                "dve",
                nc.vector.tensor_reduce(
                    mx[:], x_t[:, :KI], mybir.AxisListType.X, Alu.max,
                    apply_absolute_value=True,
                ),
            )
            s_t = sp.tile([P, 1], f32, tag="s")
            chain("dve", nc.vector.tensor_scalar(s_t[:], mx[:], inv7, None, Alu.mult))
            r_t = sp.tile([P, 1], f32, tag="r")
            chain("dve", nc.vector.reciprocal(r_t[:], s_t[:]))
            fronts[mi] = [s_t, r_t, None, None]

        def emit_front_act(mi):
            """outlier-scale / quantize for tile mi (scalar engine)."""
            x_t = x_tiles.pop(mi)
            f = fronts[mi]
            r_t = f[1]
            aos = aop.tile([P, FP], bf16, tag="aos")
            chain(
                "act",
                nc.scalar.activation(aos[:], x_t[:, KI:], Act.Copy, scale=r_t[:]),
            )
            # quantize: q+MAGIC = bf16(x*r + MAGIC) — the bf16 output convert
            # rounds to integer (ulp=1 in [184,200)); -MAGIC folds into the
            # fp8 convert after the transpose.
            q = qp.tile([P, KI], bf16)
            chain(
                "act",
                nc.scalar.activation(
                    q[:], x_t[:, :KI], Act.Copy, bias=MAGIC, scale=r_t[:]
                ),
            )
            f[2], f[3] = aos, q

        def emit_mid(mi):
            """transposes (SP HWDGE) + fp8 fixup (DVE) for tile mi."""
            s_t, r_t, aos, q = fronts.pop(mi)
            aoT = aop.tile([P, FT, P], bf16, tag="aoT")
            chain("sp", nc.sync.dma_start_transpose(aoT[:], aos[:]))
            qTb = qtp.tile([P, KT, P], bf16)
            chain("sp", nc.sync.dma_start_transpose(qTb[:], q[:]))
            qT = ftp.tile([P, KT, P], fp8)
            chain("dve", nc.vector.tensor_scalar(qT[:], qTb[:], -MAGIC, None, Alu.add))
            mids[mi] = (s_t, aoT, qT)

        # Weight DMAs: bulk on the gpsimd SWDGE ring (idle otherwise),
        # the last chunks ride the scalar HWDGE ring ahead of any
        # activation compute, ordered so chunks land just ahead of the
        # matmuls that consume them.  x loads own the SP ring.
        x_tiles[0] = emit_x_load(0)
        x_tiles[1] = emit_x_load(1)
        for j0, j1 in ((16, 24), (24, KT)):
            chain(
                "act",
                nc.scalar.dma_start(wT[:, j0:j1, :].bitcast(u8), qw8[:, j0:j1, :]),
            )
        for j0, j1 in ((0, 2), (2, 8), (8, 16)):
            chain(
                "gps",
                nc.gpsimd.dma_start(wT[:, j0:j1, :].bitcast(u8), qw8[:, j0:j1, :]),
            )
        chain("gps", nc.gpsimd.dma_start(wcs[:].bitcast(u16), wcs16[:, :, :]))
        chain(
            "gps",
            nc.gpsimd.dma_start(
                scb[:].bitcast(u16), scb16[None, :].to_broadcast((P, os_dim))
            ),
        )

        # ---------------- software-pipelined main loop ----------------
        # iteration k runs: front_dve(k+2) | mid(k+1) | front_act(k+2) |
        # x-prefetch(k+4) | matmuls(k) | evict(k-1).  The front/mid skew
        # keeps the quantize chain (~17us latency) two phases ahead of
        # the PE; per-engine emission order keeps urgent ops ahead of
        # ones that park on waits.
        emit_front_dve(0)
        emit_front_act(0)
        emit_front_dve(1)
        emit_front_act(1)
        emit_mid(0)
        x_tiles[2] = emit_x_load(2)
        x_tiles[3] = emit_x_load(3)

        prev = None
        for mi in range(MT):
            if mi + 2 < MT:
                emit_front_dve(mi + 2)
            if mi + 1 < MT:
                emit_mid(mi + 1)
            if mi + 2 < MT:
                emit_front_act(mi + 2)
            if mi + 4 < MT:
                x_tiles[mi + 4] = emit_x_load(mi + 4)

            s_t, aoT, qT = mids.pop(mi)
            # GEMM: 15 int + 2 outlier matmuls per 512 group
            psum = py.tile([P, os_dim], f32)
            for c in range(KT // 2):
                for oj in range(OJ):
                    emit_mm(
                        oj == 0,
                        psum[:, oj * 512 : (oj + 1) * 512],
                        qT[:, 2 * c : 2 * c + 2, :],
                        wT[:, 2 * c : 2 * c + 2, oj * 512 : (oj + 1) * 512],
                        start=(c == 0),
                        stop=False,
                        perf_mode=DR,
                    )
            for f in range(FT):
                for oj in range(OJ):
                    emit_mm(
                        oj == 0,
                        psum[:, oj * 512 : (oj + 1) * 512],
                        aoT[:, f, :],
                        wcs[:, f, oj * 512 : (oj + 1) * 512],
                        start=False,
                        stop=(f == FT - 1),
                    )

            if prev is not None:
                emit_evict(*prev)
            prev = (psum, s_t, mi)

        emit_evict(*prev)

    return nc


def strip_redundant_ldweights(nc, reuse_names):
    """Delete InstLdweights whose matmult reuses the already-loaded PE
    weights.  Runs after tile legalization (which pairs each matmult with
    its own InstLdweights, inserted immediately before it in the block)
    and before bacc compile.  An LDW is removed only when (a) the next PE
    instruction is a matmult flagged for reuse, (b) its weights AP is
    byte-identical to the most recent retained LDW on the PE stream, and
    (c) it carries no semaphore waits/updates."""
    import concourse.mybir as mybir

    def ap_key(pap):
        return (pap.memref, pap.offset, str(pap.ap), str(pap.dtype))

    removed = kept = 0
    for fn in nc.m.functions:
        for bb in fn.blocks:
            insts = list(bb.instructions)
            pe_next = {}  # idx -> next PE instruction
            nxt = None
            for idx in range(len(insts) - 1, -1, -1):
                pe_next[idx] = nxt
                if insts[idx].engine == mybir.EngineType.PE:
                    nxt = insts[idx]
            keep = []
            last_w = None
            changed = False
            for idx, inst in enumerate(insts):
                if isinstance(inst, mybir.InstLdweights):
                    w = ap_key(inst.ins[0])
                    mm = pe_next[idx]
                    si = inst.sync_info
                    si_clear = si is None or (
                        len(si.on_wait) == 0 and len(si.on_update) == 0
                    )
                    if (
                        isinstance(mm, mybir.InstMatmult)
                        and mm.name in reuse_names
                        and w == last_w
                        and si_clear
                    ):
                        removed += 1
                        changed = True
                        continue
                    if isinstance(mm, mybir.InstMatmult) and mm.name in reuse_names:
                        kept += 1
                    last_w = w
                keep.append(inst)
            if changed:
                bb.instructions = keep
    return removed, kept


def build_nc(ms=MS, os_dim=OS):
    import concourse.bacc as bacc
    import concourse.tile as tile

    nc = bacc.Bacc(None, target_bir_lowering=False)
    reuse_names = set()
    with tile.TileContext(nc) as tc:
        emit_core_kernel(nc, tc, ms, os_dim, reuse_names)
    removed, kept = strip_redundant_ldweights(nc, reuse_names)
    assert removed > 0, f"ldweights strip removed nothing (kept={kept})"
    nc.compile()
    return nc


def make_host_inputs(x, q_weight, scale_col, weight_cache, ind,
                     ms=MS, os_dim=OS, ncores=NCORES):
    """Shard/relayout full inputs into per-core input maps.

    Host work is index shuffling plus exact dtype relabeling (int4 codes
    -> fp8 bit patterns) and the weight_cache/scale_col division +
    bf16 rounding (weight preprocessing identical to what the device
    previously computed)."""
    ind = np.asarray(ind).astype(np.int64)
    notout = np.setdiff1d(np.arange(IN, dtype=np.int64), ind)   # 3840 sorted
    perm = np.concatenate([notout, ind])                        # dev col -> orig

    xf = np.asarray(x).reshape(M, IN).astype(np.float32, copy=False)
    xp = np.ascontiguousarray(xf[:, perm])                      # [M, IN]

    v = np.asarray(q_weight).astype(np.uint8)                   # [OUT, IN//2]
    nib = np.empty((OUT, IN), dtype=np.uint8)                   # nibble codes
    nib[:, 0::2] = v & 15
    nib[:, 1::2] = v >> 4
    nibp = nib[:, perm[:KI]]                                    # [OUT, KI]
    w8 = FP8_LUT[nibp]                                          # fp8 bits
    # device layout [p, j, o]: contraction index k = j*128 + p
    qw8 = np.ascontiguousarray(
        w8.T.reshape(KT, 128, OUT).transpose(1, 0, 2)
    )                                                           # [128, KT, OUT]

    scf = np.asarray(scale_col).reshape(-1).astype(np.float32, copy=False)
    wcT = np.asarray(weight_cache).astype(np.float32, copy=False).T  # [FP, OUT]
    wcs16 = _bf16_bits(wcT / scf[None, :]).reshape(FT, 128, OUT).transpose(1, 0, 2)
    wcs16 = np.ascontiguousarray(wcs16)                         # [128, FT, OUT]
    scb16 = _bf16_bits(scf)                                     # [OUT]

    in_maps = []
    for c in range(ncores):
        mg, og = divmod(c, OGROUPS)
        m0, o0 = mg * ms, og * os_dim
        in_maps.append(
            {
                "x": xp[m0 : m0 + ms],
                "qw8": np.ascontiguousarray(qw8[:, :, o0 : o0 + os_dim]),
                "wcs16": np.ascontiguousarray(wcs16[:, :, o0 : o0 + os_dim]),
                "scb16": np.ascontiguousarray(scb16[o0 : o0 + os_dim]),
            }
        )
    return in_maps


_NC_CACHE = {}


def kernel(x, q_weight, scale_col, weight_cache, ind, trace=False):
    from concourse.bass_utils import run_bass_kernel_spmd

    key = "full"
    if key not in _NC_CACHE:
        _NC_CACHE[key] = build_nc()
    nc = _NC_CACHE[key]

    in_maps = make_host_inputs(x, q_weight, scale_col, weight_cache, ind)
    res = run_bass_kernel_spmd(nc, in_maps, list(range(NCORES)), trace=trace)
    yfull = np.empty((M, OUT), dtype=np.float32)
    for c in range(NCORES):
        mg, og = divmod(c, OGROUPS)
        yfull[mg * MS : (mg + 1) * MS, og * OS : (og + 1) * OS] = np.asarray(
            res.results[c]["y"]
        ).astype(np.float32)
    yfull = yfull.reshape(B, S, OUT)
    if trace:
        return yfull, res
    return yfull
